# revision 65
# baseline (speedup 1.0000x reference)
"""Trainium2 Bass kernel: discounted episode returns + normalization.

reference math (full [B, T] = [4096, 8192] f32 inputs):
    ret[t] = rew[t] + 0.99 * ret[t+1] * (1 - done[t])      (reverse-time scan)
    out = (ret - ret.mean()) / (ret.std(axis=-1, ddof=1, keepdims=True) + 1e-9)

Sharding: batch axis split across 8 NeuronCores (512 rows each). The scan is
data-parallel over batch; the global mean needs one scalar AllReduce.

Final design notes (trace-driven across ~10 iterations; best measured
core-0 exec 142us, typical 150-155us, baseline was 199us):
- DVE runs the irreducible tensor_tensor_scan chain (~69us: 2.1 cyc/elem,
  fp32-state, no accelerated modes). Everything else is scheduled around
  keeping that chain stall-free and triggering the AllReduce immediately
  after it. GpSimd stays idle: it shares its SBUF port with the DVE (a 40%
  scan slowdown was measured when it streamed bulk work).
- Time chunks (512, 1024, 2048, 2048, 1536, 1024 by position, processed in
  reverse): the first-processed waves are graduated (1024, 1536, 2048...)
  so early scan consumption stays under the input DMA's ramp rate
  (~150-250GB/s for the first ~20us); small pool tiles + 8-deep pools keep
  the prefetch far ahead. Scan phase ~74us vs the 69.3us scan floor.
- The last two chunk waves (time chunks 0 and 1) use dedicated hold tiles
  for done, rew AND the a-coefficients, loaded/computed mid-pipeline, so
  the scan tail depends on nothing issued late.
- ACT runs a-coef + row-sum per unit (keeps pace with the scans); the two
  held tail waves are summed post-loop as one contiguous [0:1536) reduce
  per block, split DVE/ACT. No high_priority on that chain: hp makes the
  scheduler hoist the reduces INTO the scan stream (treated issued-first).
- Scheduler discipline learned the hard way: (a) any ACT/DMA *read* of a
  tile adds a whole-tile WAR edge that stalls the next scan writing it --
  never wire side-channels (warmup-AR inputs, extra stats passes) to ret/
  sum_parts directly; (b) the readiness-based scheduler slots deferred
  passes into critical-chain bubbles regardless of priority -- the s11
  fence before the sq passes forces the order via a data dependency.
- Sum-of-squares: folded into the loop only where WAR-safe and ACT has
  spare cadence (wave ci=2 for all blocks, ci=1 for the DVE-side blocks);
  the rest runs during the AllReduce wait window (blocks 0-1 on DVE via
  scalar_tensor_tensor accum, blocks 2-3 on ACT Square accum). The
  AllReduce-trigger chain stays engine-local: DVE reduces blocks 0/1,
  ACT accumulates blocks 2/3 AND the s11 pickup + ar_in DMA (scalar
  queue), so no cross-engine hop sits on the trigger path.
- Two dummy warm-up AllReduces: at start, and mid-pipeline at unit 11 --
  early enough that a late cc stream can never collide with the real
  AllReduce (+14.5us once), late enough to keep the rings warm (real AR
  measures 12-18us warm vs 22-29us cold). Residual 13-27us AR variance is
  inter-core arrival skew, outside the kernel's control.
- Normalize is split DVE (tensor_scalar bf16 4x) / ACT (Identity with
  per-partition scale+bias APs); output DMA per half block (8KB
  per-partition lines; quarter splits drop to ~280GB/s vs ~420GB/s).
- HBM traffic shrunk with narrow dtypes: rewards bf16 + done u8 in,
  output bf16 (upcast on the host). Scan state stays fp32, a-coefs exact
  fp32; only bf16 rounding of rewards/returns remains (~5.6e-3 vs 2e-2).
"""

from contextlib import ExitStack

import ml_dtypes
import numpy as np

import concourse.bass as bass
import concourse.mybir as mybir
import concourse.tile as tile
from concourse import bacc
from concourse.bass_utils import run_bass_kernel_spmd

F32 = mybir.dt.float32
BF16 = mybir.dt.bfloat16
U8 = mybir.dt.uint8
Alu = mybir.AluOpType
Act = mybir.ActivationFunctionType
AxL = mybir.AxisListType

DISCOUNT = 0.99
EPS = 1e-9
P = 128

N_CORES = 8
B_GLOBAL, T = 4096, 8192
B_CORE = B_GLOBAL // N_CORES
# time chunks by position; processed in reverse order. The first-processed
# waves are graduated (1024, 1536, 2048...) so early scan consumption stays
# under the input DMA's ramp rate (~150-250GB/s for the first ~20us).
CHUNKS = (512, 1024, 2048, 2048, 1536, 1024)
MAXCHUNK = max(CHUNKS)
A_HOLD_CI = (0, 1)   # chunks with held done/rew loads + precomputed a-coefs

WARMUP_AR = True
# mid-pipeline warmup AR: the real AllReduce measures 12-18us when another
# collective ran recently vs 22-29us cold. Fire it EARLY (after this many
# units) so a late cc stream can never collide with the real AllReduce
# (v13 measured a +14.5us start delay from such a collision).
AR2_BEFORE_UNIT = 11
N_SQ_DVE = 2


def _build_core_program(tc, out_ap, rew_ap, done_ap, n_cores, total_elems):
    nc = tc.nc
    B_core, T_ = rew_ap.shape
    n_blocks = B_core // P
    n_chunks = len(CHUNKS)
    starts = [sum(CHUNKS[:i]) for i in range(n_chunks)]
    assert sum(CHUNKS) == T_

    with ExitStack() as ctx:
        ret_pool = ctx.enter_context(tc.tile_pool(name="ret", bufs=1))
        rew_pool = ctx.enter_context(tc.tile_pool(name="rew", bufs=8))
        done_pool = ctx.enter_context(tc.tile_pool(name="done", bufs=8))
        a_pool = ctx.enter_context(tc.tile_pool(name="acoef", bufs=4))
        hold_pool = ctx.enter_context(tc.tile_pool(name="hold", bufs=1))
        stat_pool = ctx.enter_context(tc.tile_pool(name="stat", bufs=1))
        psum_pool = ctx.enter_context(tc.tile_pool(name="psum", bufs=1, space="PSUM"))
        dram_pool = ctx.enter_context(tc.tile_pool(name="dram", bufs=1, space="DRAM"))

        # don't-care outputs for the accum-bearing stats passes (only
        # accum_out matters). One per engine; each engine executes in order.
        act_scr = stat_pool.tile([P, MAXCHUNK], BF16, tag="act_scr",
                                 name="act_scr")
        dve_scr = stat_pool.tile([P, MAXCHUNK], BF16, tag="dve_scr",
                                 name="dve_scr")
        sum_cat = stat_pool.tile([P, n_blocks], F32)  # col b = row sums of block b
        ss_cat = stat_pool.tile([P, n_blocks], F32)   # col b = row sums of squares
        ret_tiles = []
        sum_part_tiles = []
        ss_part_tiles = []
        for b in range(n_blocks):
            ret_tiles.append(ret_pool.tile([P, T_], BF16, tag=f"ret{b}",
                                           name=f"ret{b}"))
            sum_part_tiles.append(stat_pool.tile([P, n_chunks], F32,
                                                 tag=f"smp{b}", name=f"smp{b}"))
            # col 0 <- combined tail-wave reduce; col 1 never written
            nc.vector.memset(sum_part_tiles[b][:, 1:2], 0.0)
            # ss cols: one per deferred sq pass (6 cuts cover the full row)
            ssp = stat_pool.tile([P, 6], F32, tag=f"ssp{b}", name=f"ssp{b}")
            ss_part_tiles.append(ssp)

        # stage the first chunk-row of loads before anything else (done
        # before rew: the a-coefficient chain starts from done)
        first_loads = []
        ci0 = n_chunks - 1
        lo0, hi0 = starts[ci0], starts[ci0] + CHUNKS[ci0]
        for b in range(n_blocks):
            rows = slice(b * P, (b + 1) * P)
            csz = CHUNKS[ci0]
            done_t = done_pool.tile([P, MAXCHUNK], U8, tag="done", name="done_t")
            nc.sync.dma_start(done_t[:, :csz], done_ap[rows, lo0:hi0])
            rew_t = rew_pool.tile([P, MAXCHUNK], BF16, tag="rew", name="rew_t")
            nc.sync.dma_start(rew_t[:, :csz], rew_ap[rows, lo0:hi0])
            first_loads.append((rew_t, done_t))

        a_hold = {}
        rew_hold = {}

        # preload the Sqrt activation table while ACT is idle in the DMA
        # ramp: its first real use otherwise pays a 1.28us ACT_TABLE_LOAD
        # right on the post-AllReduce critical path
        sq_warm = stat_pool.tile([1, 1], F32, tag="sq_warm", name="sq_warm")
        nc.vector.memset(sq_warm[:], 1.0)
        nc.scalar.activation(sq_warm[:], sq_warm[:], Act.Sqrt)

        # warm-up AllReduce: absorbs the collective cold-start while the
        # compute engines stream the scan phase; nothing reads ar1_out
        if WARMUP_AR and n_cores > 1:
            z = stat_pool.tile([1, 1], F32, tag="z", name="z")
            nc.vector.memset(z[:], 0.0)
            ar1_in = dram_pool.tile([1, 1], F32, tag="ar1_in", name="ar1_in")
            ar1_out = dram_pool.tile([1, 1], F32, tag="ar1_out", name="ar1_out")
            nc.gpsimd.dma_start(ar1_in[:], z[:])
            nc.gpsimd.collective_compute(
                "AllReduce", Alu.add,
                replica_groups=[list(range(n_cores))],
                ins=[ar1_in.opt()], outs=[ar1_out.opt()])

        # main pipeline: reverse time order, interleaved across blocks so
        # back-to-back DVE scans are independent (the serial carry of a
        # block is n_blocks scans back)
        unit = 0
        for ci in range(n_chunks - 1, -1, -1):
            csz = CHUNKS[ci]
            lo, hi = starts[ci], starts[ci] + csz
            for b in range(n_blocks):
                rows = slice(b * P, (b + 1) * P)
                ret_t = ret_tiles[b]
                sum_parts = sum_part_tiles[b]
                if ci == n_chunks - 1:
                    rew_t, done_t = first_loads[b]
                    rew_view = rew_t[:, :csz]
                elif ci in A_HOLD_CI:
                    rew_view = rew_hold[(ci, b)][:]
                else:
                    done_t = done_pool.tile([P, MAXCHUNK], U8, tag="done",
                                            name="done_t")
                    nc.sync.dma_start(done_t[:, :csz], done_ap[rows, lo:hi])
                    rew_t = rew_pool.tile([P, MAXCHUNK], BF16, tag="rew",
                                          name="rew_t")
                    nc.sync.dma_start(rew_t[:, :csz], rew_ap[rows, lo:hi])
                    rew_view = rew_t[:, :csz]
                # a = 0.99 - 0.99*done (exact fp32 coefficients). Unit 0 on
                # DVE (ACT's first op pays the activation-table load); tail
                # chunks precomputed mid-loop; the rest on ACT at priority 0.
                if ci in A_HOLD_CI:
                    a_view = a_hold[(ci, b)][:]
                else:
                    a_t = a_pool.tile([P, MAXCHUNK], F32, tag="a", name="a_t")
                    a_view = a_t[:, :csz]
                    if unit == 0:
                        nc.vector.tensor_scalar(a_view, done_t[:, :csz],
                                                -DISCOUNT, DISCOUNT,
                                                Alu.mult, Alu.add)
                    else:
                        with tc.high_priority():
                            nc.scalar.activation(a_view, done_t[:, :csz],
                                                 Act.Copy, bias=DISCOUNT,
                                                 scale=-DISCOUNT)
                # reversed scan: state = a*state + rew, columns hi-1 .. lo
                init = 0.0 if ci == n_chunks - 1 else ret_t[:, hi:hi + 1]
                nc.vector.tensor_tensor_scan(
                    ret_t[:, lo:hi][:, ::-1], a_view[:, ::-1],
                    rew_view[:, ::-1],
                    init, Alu.mult, Alu.add)
                # row sums feed the AllReduce. Early waves on ACT (keeps
                # pace); the last two waves are reduced post-loop split
                # DVE/ACT (in-loop tail sums serialize behind ACT's backlog:
                # its accum ops carry a 0.28us READ_ACCUMULATOR tax each,
                # so ACT trails the scans by ~5us at the ci=2 wave).
                if ci not in A_HOLD_CI:
                    nc.scalar.activation(act_scr[:, :csz], ret_t[:, lo:hi],
                                         Act.Copy,
                                         accum_out=sum_parts[:, ci:ci + 1])
                # NO in-loop sum-of-squares at all: each ACT accum op costs
                # its pass time PLUS a 0.28us ACTIVATION_READ_ACCUMULATOR,
                # so even the "spare cadence" ci=2 wave overflows (4.56 >
                # 4.33us/unit), backlogging ACT into the trigger chain; and
                # in-loop ci=1 squares add WAR edges that stall the ci=0
                # scans ~1.7us each. All squares run in the AR wait window.
                unit += 1
                # second warmup AR, via a dead-end tile (a DMA reading
                # sum_parts/ret directly adds a WAR edge stalling later
                # writers of those tiles)
                if WARMUP_AR and n_cores > 1 and unit == AR2_BEFORE_UNIT:
                    ar2_src = stat_pool.tile([1, 1], F32, tag="ar2_src",
                                             name="ar2_src")
                    nc.scalar.activation(ar2_src[:],
                                         sum_parts[0:1, ci:ci + 1], Act.Copy)
                    ar2_in = dram_pool.tile([1, 1], F32, tag="ar2_in",
                                            name="ar2_in")
                    ar2_out = dram_pool.tile([1, 1], F32, tag="ar2_out",
                                             name="ar2_out")
                    nc.gpsimd.dma_start(ar2_in[:], ar2_src[:])
                    nc.gpsimd.collective_compute(
                        "AllReduce", Alu.add,
                        replica_groups=[list(range(n_cores))],
                        ins=[ar2_in.opt()], outs=[ar2_out.opt()])
            # held loads + a-coefs for the tail chunks, on otherwise-idle
            # ACT/DMA time. Issued after the THIRD wave's loads: issuing
            # them earlier parks 2.25MB of not-yet-needed traffic in the
            # middle of the input stream and stalls wave 3's scans ~3us
            # (the holds are only consumed from t~75).
            if ci == n_chunks - 3:
                for hci in A_HOLD_CI:
                    hsz = CHUNKS[hci]
                    hlo = starts[hci]
                    for hb in range(n_blocks):
                        hrows = slice(hb * P, (hb + 1) * P)
                        dh = hold_pool.tile([P, hsz], U8, tag=f"dh{hci}_{hb}",
                                            name=f"dh{hci}_{hb}")
                        nc.sync.dma_start(dh[:], done_ap[hrows, hlo:hlo + hsz])
                        rh = hold_pool.tile([P, hsz], BF16,
                                            tag=f"rh{hci}_{hb}",
                                            name=f"rh{hci}_{hb}")
                        nc.sync.dma_start(rh[:], rew_ap[hrows, hlo:hlo + hsz])
                        rew_hold[(hci, hb)] = rh
                        ah = hold_pool.tile([P, hsz], F32, tag=f"ah{hci}_{hb}",
                                            name=f"ah{hci}_{hb}")
                        nc.scalar.activation(ah[:], dh[:], Act.Copy,
                                             bias=DISCOUNT, scale=-DISCOUNT)
                        a_hold[(hci, hb)] = ah

        # ---- global-sum AllReduce critical path. NOTE: no high_priority
        # here -- hp hoisted these reduces INTO the scan stream in v12 (the
        # scheduler treats hp as issued-first and slotted them between the
        # tail scans). At natural (late) priority they sort after the scans.
        # The held tail waves (time [0:1536)) are contiguous: one reduce per
        # block, split DVE/ACT so both engines chew the tail in parallel.
        ones_col = stat_pool.tile([P, 1], F32)
        nc.vector.memset(ones_col[:], 1.0)
        psum_t = psum_pool.tile([1, n_blocks], F32, tag="psum_t", name="psum_t")
        tail_hi = starts[max(A_HOLD_CI)] + CHUNKS[max(A_HOLD_CI)]
        for b in range(n_blocks):
            if b < n_blocks // 2:
                nc.vector.tensor_reduce(sum_part_tiles[b][:, 0:1],
                                        ret_tiles[b][:, 0:tail_hi], AxL.X,
                                        Alu.add)
                nc.vector.tensor_reduce(sum_cat[:, b:b + 1],
                                        sum_part_tiles[b][:], AxL.X, Alu.add)
            else:
                # blocks 2/3 stay entirely on ACT (tail-accum AND the
                # per-block total) so the DVE never waits on ACT here
                nc.scalar.activation(act_scr[:, :tail_hi],
                                     ret_tiles[b][:, 0:tail_hi], Act.Copy,
                                     accum_out=sum_part_tiles[b][:, 0:1])
                nc.scalar.activation(act_scr[:, :n_chunks],
                                     sum_part_tiles[b][:], Act.Copy,
                                     accum_out=sum_cat[:, b:b + 1])
        nc.tensor.matmul(psum_t[:], ones_col[:], sum_cat[:], start=True,
                         stop=True)
        s11 = stat_pool.tile([1, 1], F32)
        nc.scalar.activation(act_scr[0:1, :n_blocks], psum_t[:], Act.Copy,
                             accum_out=s11[:])
        g_sb = stat_pool.tile([1, 1], F32)
        if n_cores > 1:
            ar_in = dram_pool.tile([1, 1], F32, tag="ar_in", name="ar_in")
            ar_out = dram_pool.tile([1, 1], F32, tag="ar_out",
                                    name="ar_out")
            # triggered from the scalar queue: ACT just wrote s11, so no
            # cross-engine semaphore hop sits on the AR critical path
            nc.scalar.dma_start(ar_in[:], s11[:])
            nc.gpsimd.collective_compute(
                "AllReduce", Alu.add,
                replica_groups=[list(range(n_cores))],
                ins=[ar_in.opt()], outs=[ar_out.opt()])
            # fetch on the gpsimd queue: it sits right after the collective
            # on the same engine, so it fires the moment the AR completes
            nc.gpsimd.dma_start(g_sb[:], ar_out[:])
        else:
            loc = dram_pool.tile([1, 1], F32, tag="loc", name="loc")
            nc.sync.dma_start(loc[:], s11[:])
            nc.sync.dma_start(g_sb[:], loc[:])

        # ---- sum-of-squares in MAXCHUNK passes, during the AR wait:
        # blocks 0..N_SQ_DVE-1 on DVE (scalar_tensor_tensor + accum), the
        # rest on ACT (Square + accum). The fence below reads s11 and
        # writes dve_scr, so the DVE's first sq pass (WAW on dve_scr)
        # cannot be scheduled ahead of the AllReduce-trigger chain -- the
        # scheduler orders by readiness estimates, and hp alone lost to
        # them in v9/v11 (4.4us sq passes slotted into chain bubbles).
        # fence reads a tile written ONLY by the DVE chain (block 1's tail
        # reduce), so the DVE's deferred sq passes sort after its
        # trigger-chain reduces but wait for nothing cross-engine -- a
        # sum_cat read here would also wait on ACT's block-2/3 accums
        # (~2-3us later) since tile deps are whole-tile
        nc.vector.tensor_scalar_add(dve_scr[0:1, 0:1],
                                    sum_part_tiles[1][0:1, 0:1], 0.0)
        # full-row coverage in MAXCHUNK-bounded cuts, blocks 0/1 on DVE,
        # blocks 2/3 on ACT
        sq_cuts = [(0, starts[1]), (starts[1], starts[2])] + [
            (c, min(c + MAXCHUNK, T_)) for c in range(starts[2], T_, MAXCHUNK)]
        for b in range(n_blocks):
            ret_t = ret_tiles[b]
            ssp = ss_part_tiles[b]
            for q, (qlo, qhi) in enumerate(sq_cuts):
                cols = slice(qlo, qhi)
                csz_q = qhi - qlo
                # block 2's [1536:3584) cut also goes to DVE: the fence fix
                # lets DVE start ~6.5us before ACT (whose first sq waits
                # s11), so DVE carries one extra cut to balance both ss
                # finish times at ~109.5
                if b < N_SQ_DVE or (b == N_SQ_DVE and q == 2):
                    nc.vector.scalar_tensor_tensor(
                        dve_scr[:, :csz_q], ret_t[:, cols], 1.0,
                        ret_t[:, cols], Alu.mult, Alu.mult,
                        accum_out=ssp[:, q:q + 1])
                else:
                    nc.scalar.activation(act_scr[:, :csz_q], ret_t[:, cols],
                                         Act.Square,
                                         accum_out=ssp[:, q:q + 1])
        for b in range(n_blocks):
            nc.vector.tensor_reduce(ss_cat[:, b:b + 1], ss_part_tiles[b][:],
                                    AxL.X, Alu.add)

        # ---- per-row 1/(std+eps): independent of the AllReduce ----
        sum_sq = stat_pool.tile([P, n_blocks], F32)
        nc.vector.tensor_tensor(sum_sq[:], sum_cat[:], sum_cat[:], Alu.mult)
        u = stat_pool.tile([P, n_blocks], F32)
        nc.vector.scalar_tensor_tensor(u[:], sum_sq[:], -1.0 / T_, ss_cat[:],
                                       Alu.mult, Alu.add)  # ss - sum^2/T
        stdv = stat_pool.tile([P, n_blocks], F32)
        nc.scalar.activation(stdv[:], u[:], Act.Sqrt, scale=1.0 / (T_ - 1))
        nc.vector.tensor_scalar_add(stdv[:], stdv[:], EPS)
        inv_cat = stat_pool.tile([P, n_blocks], F32)
        nc.vector.reciprocal(inv_cat[:], stdv[:])

        # The AR result comes back as a [1,1] DMA to partition 0, then a
        # ones[1,128] matmul replicates it across partitions in PSUM (~25ns)
        # -- cheaper than a 128-packet partition-broadcast DMA.
        ones_row = stat_pool.tile([1, P], F32)
        nc.vector.memset(ones_row[:], 1.0)
        psum_g = psum_pool.tile([P, 1], F32, tag="psum_g", name="psum_g")
        nc.tensor.matmul(psum_g[:], ones_row[:], g_sb[:], start=True, stop=True)

        negb_cat = stat_pool.tile([P, n_blocks], F32)
        nc.vector.tensor_scalar(negb_cat[:], inv_cat[:], psum_g[:, 0:1],
                                -1.0 / total_elems, Alu.mult, Alu.mult)

        # ---- normalize in place, stream out per half block. Split across
        # DVE (tensor_scalar, bf16 4x) and ACT (Identity with per-partition
        # scale/bias APs) so the serial normalize phase halves.
        half = T_ // 2
        for b in range(n_blocks):
            rows = slice(b * P, (b + 1) * P)
            ret_t = ret_tiles[b]
            for h in range(2):
                cols = slice(h * half, (h + 1) * half)
                if b < n_blocks // 2:
                    nc.vector.tensor_scalar(ret_t[:, cols], ret_t[:, cols],
                                            inv_cat[:, b:b + 1],
                                            negb_cat[:, b:b + 1],
                                            Alu.mult, Alu.add)
                else:
                    nc.scalar.activation(ret_t[:, cols], ret_t[:, cols],
                                         Act.Identity,
                                         bias=negb_cat[:, b:b + 1],
                                         scale=inv_cat[:, b:b + 1])
                nc.sync.dma_start(out_ap[rows, cols], ret_t[:, cols])


_NC_CACHE = None


def _get_nc():
    global _NC_CACHE
    if _NC_CACHE is None:
        nc = bacc.Bacc("TRN2", target_bir_lowering=False, debug=False,
                       enable_asserts=False, num_devices=N_CORES)
        rew = nc.dram_tensor("rewards", [B_CORE, T], BF16, kind="ExternalInput")
        done = nc.dram_tensor("done_flags", [B_CORE, T], U8, kind="ExternalInput")
        out = nc.dram_tensor("out", [B_CORE, T], BF16, kind="ExternalOutput")
        with tile.TileContext(nc) as tc:
            _build_core_program(tc, out.ap(), rew.ap(), done.ap(),
                                n_cores=N_CORES, total_elems=B_GLOBAL * T)
        nc.compile()
        _NC_CACHE = nc
    return _NC_CACHE


def run_sharded(rewards, done_flags, trace=False, **kwargs):
    """Run the SPMD kernel; returns (full_output, BassKernelResults)."""
    nc = _get_nc()
    rew16 = rewards.astype(ml_dtypes.bfloat16)
    done8 = done_flags.astype(np.uint8)
    in_maps = []
    for c in range(N_CORES):
        rows = slice(c * B_CORE, (c + 1) * B_CORE)
        in_maps.append({
            "rewards": np.ascontiguousarray(rew16[rows]),
            "done_flags": np.ascontiguousarray(done8[rows]),
        })
    res = run_bass_kernel_spmd(nc, in_maps, core_ids=list(range(N_CORES)),
                               trace=trace, **kwargs)
    full = np.concatenate(
        [res.results[c]["out"].astype(np.float32) for c in range(N_CORES)],
        axis=0)
    return full, res


def kernel(rewards, done_flags):
    out, _ = run_sharded(rewards, done_flags, trace=False)
    return out
